# revision 9
# baseline (speedup 1.0000x reference)
"""Multi-head attention (B=2, S=2048, H=1024, 16 heads) on 8 NeuronCores, v2.

Tensor-parallel: 2 heads per core, host sums the 8 partial outputs.

Measured ~225us vs the 328.9us v1 baseline.  Differences vs v1:
  * hs is transposed AND cast to bf16 on the host: no PE transposes for
    the QKV inputs, half the input DMA bytes.
  * All matmul operands bf16 (PSUM stays f32).  PT holds a whole
    (batch,head,1024q) block of exp(scores) so PV trails scores by one
    full iteration and never waits on the Act engine's exp.
  * Normalization moved BEFORE the output projection: a ones column (64)
    in Vn makes P@V emit the softmax denominator as output row 64; the
    denominator row is copied to partition 0 (plain DVE ops handle
    partition offsets; custom-DVE/gpsimd ops require base 0), inverted
    with the fast approx reciprocal, broadcast across 64 partitions on
    gpsimd, and one DVE multiply normalizes ctx while casting it out of
    PSUM; dense is then a single matmul per chunk (v1 ran two plus two
    vector ops per output tile).
  * Emission is software-pipelined and Act-paced: exp on the Act engine
    is the per-iteration rate limiter (~1.04us per [128,1024] tile), so
    scores+PV leave ~3us/iteration of PE slack that is filled with
    batch-1 QKV units and dense/output chunks.  Filler bursts are kept
    ~3.4us or smaller: anything bigger stalls the Act engine; anything
    finer adds per-boundary semaphore cost (measured both ways).
  * PE gaps under ~300ns do not drop the PE p-state clock; larger ones
    cost ~3us of half-clock ramp, so the schedule optimizes for short
    gaps rather than none.
  * fp8 Q/K was tried and REJECTED: softmax does not damp relative score
    noise (ctx shrinks as fast as the error), measured 10x error blowup.
  * bf16 partial outputs (half the output DMA bytes), summed f32 on host.

PSUM budget (8 banks): scores/qkv tag "mm" [128,1024]f32 x2 = 4, dense +
V-transpose tag "dn2" [128,512]f32 x2 = 2, PV accumulators pva/pvb
[65,512]f32 = 2.
"""

import os
import sys
import types

sys.path.insert(0, "/opt/trn_rl_repo")

import numpy as np
import ml_dtypes

BF16 = ml_dtypes.bfloat16
FP8N = ml_dtypes.float8_e4m3


def _install_ntff_shim():
    if "antenv.axon_hooks" in sys.modules:
        return
    try:
        from trn_agent_boot.trn_boot import _ntff_profile_via_ctypes
        so = "/opt/axon/libaxon_pjrt.so"
        if not os.path.exists(so):
            return
        hook = _ntff_profile_via_ctypes(so)
        mod = types.ModuleType("antenv.axon_hooks")
        mod.get_axon_ntff_profile_hook = lambda: hook
        mod.set_axon_ntff_profile_hook = lambda h: None
        sys.modules["antenv.axon_hooks"] = mod
    except Exception:
        pass


_install_ntff_shim()

import concourse.bass as bass
import concourse.mybir as mybir
import concourse.tile as tile
from concourse import bacc
from concourse.bass_utils import run_bass_kernel_spmd
from concourse.masks import make_identity

F32 = mybir.dt.float32
F32R = mybir.dt.float32r
FP8 = mybir.dt.float8e4
DR = mybir.MatmulPerfMode.DoubleRow
BF16D = mybir.dt.bfloat16
EXP = mybir.ActivationFunctionType.Exp
MUL = mybir.AluOpType.mult

B, S, HID = 2, 2048, 1024
HEADS, D = 16, 64
SEQ = B * S                      # 4096
NCORES = 8
HPC = HEADS // NCORES            # 2 heads per core
CW = HPC * D                     # 128
NHB = HID // 128                 # 8
WSEQ = 1024                      # phase-1 window
NWIN = SEQ // WSEQ               # 4
NCH = SEQ // 128                 # 32 seq chunks
NKT = S // 128                   # 16 k chunks per batch


def build_nc():
    nc = bacc.Bacc("TRN2", target_bir_lowering=False, debug=False,
                   num_devices=NCORES)

    hsT = nc.dram_tensor("hsT", [128, NHB, SEQ], BF16D, kind="ExternalInput")
    wq = nc.dram_tensor("wq", [128, NHB, CW], BF16D, kind="ExternalInput")
    wk = nc.dram_tensor("wk", [128, NHB, CW], BF16D, kind="ExternalInput")
    wv = nc.dram_tensor("wv", [128, NHB, CW], BF16D, kind="ExternalInput")
    bq = nc.dram_tensor("bq", [CW, 1], F32, kind="ExternalInput")
    bk = nc.dram_tensor("bk", [CW, 1], F32, kind="ExternalInput")
    wd = nc.dram_tensor("wd", [CW, HID], BF16D, kind="ExternalInput")
    out = nc.dram_tensor("out", [SEQ, HID], BF16D, kind="ExternalOutput")

    with tile.TileContext(nc) as tc:
        with (
            tc.tile_pool(name="persist", bufs=1) as pp,
            tc.tile_pool(name="hsw", bufs=3) as hwp,
            tc.tile_pool(name="vtw", bufs=2) as vwp,
            tc.tile_pool(name="ptp", bufs=2) as ptp,
            tc.tile_pool(name="bcp", bufs=2) as bcp,
            tc.tile_pool(name="outst", bufs=4) as osp,
            tc.tile_pool(name="ps_mm", bufs=2,
                         space=bass.MemorySpace.PSUM) as pmm,
            tc.tile_pool(name="ps_pv", bufs=1,
                         space=bass.MemorySpace.PSUM) as ppv,
            tc.tile_pool(name="ps_dn", bufs=2,
                         space=bass.MemorySpace.PSUM) as pdn,
        ):
            # ---------------- persistent tiles ------------------------
            ident = pp.tile([128, 128], F32)
            make_identity(nc, ident[:])
            identr_t = pp.tile([128, 128], F32R)
            nc.vector.tensor_copy(identr_t[:], ident[:])
            identr = identr_t[:]

            wq_sb = pp.tile([128, NHB, CW], BF16D)
            wk_sb = pp.tile([128, NHB, CW], BF16D)
            wv_sb = pp.tile([128, NHB, CW], BF16D)
            bq_sb = pp.tile([CW, 1], F32)
            bk_sb = pp.tile([CW, 1], F32)
            wd_sb = pp.tile([CW, HID], BF16D)

            def emit_weight_dmas():
                nc.scalar.dma_start(wv_sb[:], wv[:])
                nc.scalar.dma_start(bq_sb[:], bq[:])
                nc.scalar.dma_start(bk_sb[:], bk[:])
                nc.scalar.dma_start(wd_sb[:], wd[:])

            # QTz/KTz: [128, head, seq]; rows 0-63 = head data, rows
            # 64-127 zero so scores contract over a full 128 partitions
            # (keeps the PE in 128x128 mode throughout).
            QTz = pp.tile([128, HPC, SEQ], BF16D)
            KTz = pp.tile([128, HPC, SEQ], BF16D)
            # Vn: [k%128, chunk, head, 66]; cols 0-63 = v dims, col 64 = 1.0
            # so P@V emits the softmax denominator as out row 64.
            Vn = pp.tile([128, NCH, HPC, 66], BF16D)
            ctxT = pp.tile([128, SEQ], BF16D)

            nc.gpsimd.memset(QTz[D:128, :, :], 0.0)
            nc.gpsimd.memset(KTz[D:128, :, :], 0.0)
            nc.gpsimd.memset(Vn[:, :, :, 64:65], 1.0)

            hs_tiles = {}

            def emit_hs_dma(w, split=1):
                t = hwp.tile([128, NHB, WSEQ], BF16D, tag="hs",
                             name=f"hsw{w}")
                wsl = slice(w * WSEQ, (w + 1) * WSEQ)
                if split == 1:
                    nc.sync.dma_start(t[:], hsT[:, :, wsl])
                else:
                    # split across DMA queues so the first window lands fast
                    engs = (nc.sync, nc.scalar)
                    hbs = NHB // split
                    for i in range(split):
                        hsl = slice(i * hbs, (i + 1) * hbs)
                        engs[i % len(engs)].dma_start(t[:, hsl, :],
                                                      hsT[:, hsl, wsl])
                hs_tiles[w] = t

            def emit_ph1_half(w, tgt, qh2):
                """Half of one QKV target for one 1024-seq window:
                one 512-seq slice (8 matmuls + epilogue), ~1.7us PE."""
                hsw = hs_tiles[w]
                s0 = w * WSEQ + qh2 * 512
                wsl = slice(s0, s0 + 512)
                qsl = slice(qh2 * 512, (qh2 + 1) * 512)
                wsb = (wq_sb, wk_sb, wv_sb)[tgt]
                ps = pmm.tile([128, WSEQ], F32, tag="mm")
                sl = slice(0, 512)
                for hb in range(NHB):
                    nc.tensor.matmul(
                        ps[:, sl], wsb[:, hb, :], hsw[:, hb, qsl],
                        start=(hb == 0), stop=(hb == NHB - 1))
                if tgt < 2:
                    dst, bias = ((QTz, bq_sb), (KTz, bk_sb))[tgt]
                    for h in range(HPC):
                        nc.vector.tensor_scalar_add(
                            dst[0:D, h, wsl], ps[h * D:(h + 1) * D, sl],
                            bias[h * D:(h + 1) * D, 0:1])
                else:
                    # v: copy to SBUF, PE-transpose 128-blocks (f32r),
                    # store natural-layout bf16 into Vn (cols 0-63)
                    vt = vwp.tile([128, 512], F32R, tag="vt")
                    nc.vector.tensor_copy(vt[:], ps[:, sl])
                    trp = pdn.tile([128, 512], F32, tag="dn2")
                    for i in range(4):
                        nc.tensor.transpose(
                            trp[:, i * 128:(i + 1) * 128].bitcast(F32R),
                            vt[:, i * 128:(i + 1) * 128],
                            identr)
                    ch0 = s0 // 128
                    nc.vector.tensor_copy(
                        Vn[:, ch0:ch0 + 4, :, 0:64],
                        trp[:].rearrange("p (c h d) -> p c h d",
                                         c=4, h=HPC))

            def emit_ph1_unit(w, tgt):
                emit_ph1_half(w, tgt, 0)
                emit_ph1_half(w, tgt, 1)

            # attention iterations: batch-major, then q-half, then head,
            # so each (b, qh) dense half is ready two iterations later
            iters = [(b, h, qh) for b in range(B) for qh in range(2)
                     for h in range(HPC)]
            pt_tiles = {}
            pv_tiles = {}

            def emit_scores_step(it, kt):
                b, h, qh = iters[it]
                if kt == 0:
                    pt_tiles[it] = ptp.tile([128, NKT, WSEQ], BF16D,
                                            tag="pt", name=f"pt{it}")
                pt = pt_tiles[it]
                ksl = slice(b * S + kt * 128, b * S + (kt + 1) * 128)
                q0 = b * S + qh * WSEQ
                st = pmm.tile([128, WSEQ], F32, tag="mm")
                for qs2 in range(2):
                    nc.tensor.matmul(
                        st[:, qs2 * 512:(qs2 + 1) * 512],
                        KTz[:, h, ksl],
                        QTz[:, h, q0 + qs2 * 512:q0 + (qs2 + 1) * 512],
                        start=True, stop=True)
                nc.scalar.activation(pt[:, kt, :], st[:], EXP, scale=0.125)

            def emit_pv_step(it, kt, qsls=(0, 1)):
                b, h, qh = iters[it]
                pt = pt_tiles[it]
                if it not in pv_tiles:
                    pv_tiles[it] = (
                        ppv.tile([D + 1, 512], F32, tag="pva",
                                 name=f"pva{it}"),
                        ppv.tile([D + 1, 512], F32, tag="pvb",
                                 name=f"pvb{it}"))
                pva, pvb = pv_tiles[it]
                ch = b * NKT + kt
                for qsl in qsls:
                    pvh = (pva, pvb)[qsl]
                    nc.tensor.matmul(
                        pvh[:], Vn[:, ch, h, 0:65],
                        pt[:, kt, qsl * 512:(qsl + 1) * 512],
                        start=(kt == 0), stop=(kt == NKT - 1))

            def emit_pv_epilogue(it, qsls=(0, 1)):
                b, h, qh = iters[it]
                pva, pvb = pv_tiles[it]
                for qsl in qsls:
                    pvh = (pva, pvb)[qsl]
                    cols = slice(b * S + qh * WSEQ + qsl * 512,
                                 b * S + qh * WSEQ + (qsl + 1) * 512)
                    # only base-0-safe ops touch the ISA-lowered paths:
                    # plain DVE ops handle partition offsets; custom DVE
                    # and gpsimd ops require base-0 inputs
                    cr = bcp.tile([D + 1, 512], F32, tag="cr")
                    nc.vector.tensor_copy(cr[:], pvh[:])
                    dn0 = bcp.tile([1, 512], F32, tag="dn0")
                    nc.vector.tensor_copy(dn0[:], cr[D:D + 1, :])
                    dn1 = bcp.tile([1, 512], F32, tag="dn1")
                    nc.vector.reciprocal_approx_fast(dn1[:], dn0[:])
                    rc = bcp.tile([D, 512], F32, tag="rc")
                    nc.gpsimd.partition_broadcast(rc[:], dn1[:], channels=D)
                    nc.vector.tensor_tensor(
                        ctxT[h * D:(h + 1) * D, cols],
                        cr[0:D, :], rc[:], MUL)
                if 1 in qsls:
                    del pt_tiles[it], pv_tiles[it]

            def emit_dense_chunk(b, qh, ch, eng):
                s0 = b * S + qh * WSEQ + ch * 128
                ssl = slice(s0, s0 + 128)
                ob = osp.tile([128, HID], BF16D, tag="ob")
                for nt in range(2):
                    nsl = slice(nt * 512, (nt + 1) * 512)
                    dpt = pdn.tile([128, 512], F32, tag="dn2")
                    nc.tensor.matmul(dpt[:], ctxT[:, ssl],
                                     wd_sb[:, nsl], start=True, stop=True)
                    if eng:
                        # tail chunks: Act is idle after the last exp, so
                        # its copies run parallel to DVE's epilogue chains
                        nc.scalar.copy(ob[:, nsl], dpt[:])
                    else:
                        nc.vector.tensor_copy(ob[:, nsl], dpt[:])
                # tail chunks (eng=1) split the drain across the sync and
                # the idle gpsimd SWDGE queues
                q = (nc.sync, nc.gpsimd)[ch % 2] if eng else nc.sync
                q.dma_start(out[ssl, :], ob[:])

            def emit_dense_half(b, qh):
                for ch in range(WSEQ // 128):
                    emit_dense_chunk(b, qh, ch, ch % 2)

            # ---------------- emission schedule -----------------------
            nc.sync.dma_start(wq_sb[:], wq[:])
            nc.sync.dma_start(wk_sb[:], wk[:])
            emit_hs_dma(0, split=4)
            emit_weight_dmas()
            emit_hs_dma(1, split=2)
            for tgt in range(3):
                emit_ph1_unit(0, tgt)
            emit_hs_dma(2)
            for tgt in range(3):
                emit_ph1_unit(1, tgt)
            emit_hs_dma(3)

            def ph1u(w, tgt):
                return lambda: emit_ph1_unit(w, tgt)

            # PE filler units spread across each iteration's kt loop
            fillers = {
                0: [ph1u(2, 0), ph1u(2, 1), ph1u(2, 2)],
                1: [ph1u(3, 0), ph1u(3, 1)],
                2: [ph1u(3, 2)],
                3: [lambda: emit_dense_half(0, 0)],
                5: [lambda: emit_dense_half(0, 1)],
                7: [lambda: emit_dense_half(1, 0)],
            }
            fire_at = {3: 0, 8: 1, 13: 2}

            for it in range(len(iters)):
                fl = fillers.get(it, [])
                for kt in range(NKT):
                    emit_scores_step(it, kt)
                    if it > 0:
                        emit_pv_step(it - 1, kt)
                    fi = fire_at.get(kt)
                    if fi is not None and fi < len(fl):
                        fl[fi]()
                if it > 0:
                    emit_pv_epilogue(it - 1)
            # tail: qsl-serialized PV so the first epilogue chain runs
            # while the second half still accumulates on the PE
            last = len(iters) - 1
            for kt in range(NKT):
                emit_pv_step(last, kt, qsls=(0,))
            emit_pv_epilogue(last, qsls=(0,))
            for kt in range(NKT):
                emit_pv_step(last, kt, qsls=(1,))
            for ch in range(4):
                emit_dense_chunk(1, 1, ch, 0)
            emit_pv_epilogue(last, qsls=(1,))
            for ch in range(4, 8):
                emit_dense_chunk(1, 1, ch, 0)

    nc.compile()
    return nc


_NC_CACHE = None


def get_nc():
    global _NC_CACHE
    if _NC_CACHE is None:
        _NC_CACHE = build_nc()
    return _NC_CACHE


def make_in_maps(hidden_states, w_qkv, b_qkv, w_dense):
    hs = np.ascontiguousarray(
        np.asarray(hidden_states, dtype=np.float32).reshape(SEQ, HID))
    w_qkv = np.asarray(w_qkv, dtype=np.float32)
    b_qkv = np.asarray(b_qkv, dtype=np.float32)
    w_dense = np.asarray(w_dense, dtype=np.float32)
    # hs^T in [partition, hid-chunk, seq] layout, bf16
    hsT = np.ascontiguousarray(
        hs.T.reshape(NHB, 128, SEQ).transpose(1, 0, 2)).astype(BF16)
    # reference column order: per head [q_h | k_h | v_h] blocks of D
    wq_cols = np.concatenate(
        [np.arange(h * 3 * D, h * 3 * D + D) for h in range(HEADS)])
    wk_cols = wq_cols + D
    wv_cols = wq_cols + 2 * D
    in_maps = []
    for c in range(NCORES):
        sel = slice(c * CW, (c + 1) * CW)

        def wprep(cols):
            w = w_qkv[:, cols[sel]]                      # [1024, 128]
            return np.ascontiguousarray(
                w.reshape(NHB, 128, CW).transpose(1, 0, 2)).astype(BF16)

        in_maps.append({
            "hsT": hsT,
            "wq": wprep(wq_cols),
            "wk": wprep(wk_cols),
            "wv": wprep(wv_cols),
            "bq": np.ascontiguousarray(
                b_qkv[wq_cols[sel]].reshape(CW, 1)).astype(np.float32),
            "bk": np.ascontiguousarray(
                b_qkv[wk_cols[sel]].reshape(CW, 1)).astype(np.float32),
            "wd": np.ascontiguousarray(w_dense[sel, :]).astype(BF16),
        })
    return in_maps


def run(hidden_states, w_qkv, b_qkv, w_dense, b_dense, trace=False):
    nc = get_nc()
    in_maps = make_in_maps(hidden_states, w_qkv, b_qkv, w_dense)
    res = run_bass_kernel_spmd(nc, in_maps, core_ids=list(range(NCORES)),
                               trace=trace)
    acc = res.results[0]["out"].astype(np.float32)
    for c in range(1, NCORES):
        acc = acc + res.results[c]["out"].astype(np.float32)
    # v-bias commutes through attention into the dense layer; add with
    # the dense bias on the host.
    b_qkv = np.asarray(b_qkv, dtype=np.float32)
    b_v = np.concatenate(
        [b_qkv[h * 3 * D + 2 * D:h * 3 * D + 3 * D] for h in range(HEADS)])
    acc = acc + (b_v @ np.asarray(w_dense, dtype=np.float32)
                 + np.asarray(b_dense, dtype=np.float32))
    return acc.reshape(B, S, HID).astype(np.float32), res


def kernel(hidden_states, w_qkv, b_qkv, w_dense, b_dense):
    out, _ = run(hidden_states, w_qkv, b_qkv, w_dense, b_dense,
                 trace=bool(os.environ.get("BASS_TRACE")))
    return out
